# revision 1
# baseline (speedup 1.0000x reference)
"""Trainium2 Bass kernel for nn_Attention_48206712930624.

Dense transformer block: LayerNorm -> QKV proj -> 8-head attention
(head_dim = 512) -> output projection.  B=4, S=2048, D=512, H=8.

Sharding: tensor-parallel over heads -- each of the 8 NeuronCores computes
one head end-to-end (LN duplicated), producing a partial output projection
Y_h = (P_h @ V_h) @ o_w_h (un-normalized) plus the softmax denominators
l_h.  The host combines:  out = sum_h Y_h / l_h + const.

Device-side design notes:
  * All big matmuls run in float32r (TF32-like, full PE rate at N=512,
    ~1.5e-4 rel err) with fp32 PSUM accumulation.
  * Scores are computed TRANSPOSED (S^T[k,q] = k^T.T @ q^T) so softmax's
    exp is orientation-agnostic (ACT elementwise) and P^T lands directly
    in the layout att@V needs (k on partitions).  Row sums l are computed
    with a ones-vector matmul.  No max-subtraction (logits ~ N(0,1);
    folded scale keeps exp well within fp32 range).
  * LayerNorm scale/bias and the attention scale are folded into the
    weights on the host; v-bias and o_b fold into a constant row added on
    the host.  rstd = exp(-0.5*ln(var+eps)) and DVE-side bias adds keep
    the whole kernel on ONE ACT table set (natural_log_exp_and_others).
  * Scores use paired PSUM banks so each exp covers [128,1024].
  * Batch-level software pipeline: batch b+1's LayerNorm+transpose is
    emitted inside batch b's attention phase (right after the last q
    projection frees the xhT buffer slot) so the PE never waits on the
    serial LN chain at batch boundaries.
"""

import sys

import numpy as np

for _p in ("/opt/trn_rl_repo", "/root/.axon_site/_ro/trn_rl_repo"):
    if _p not in sys.path:
        sys.path.append(_p)

import concourse.bacc as bacc
import concourse.mybir as mybir
import concourse.tile as tile
from concourse.bass_utils import run_bass_kernel_spmd
from concourse.masks import make_identity

# Steer the ACT-table-load placement pass to the one set that holds every
# function this kernel uses (ln, exp, copy, identity), so the whole kernel
# runs on a single table load instead of thrashing between per-function
# sets.  Only the pass's view is doctored; runtime tables are untouched.
_ONE_SET = "natural_log_exp_and_others"
_orig_get_act_tables = bacc.get_activation_tables


def _patched_get_act_tables(arch):
    t = _orig_get_act_tables(arch)
    af = mybir.ActivationFunctionType
    strip = {af.Ln, af.Exp, af.Copy, af.Identity}
    return {
        name: (set(fns) if name == _ONE_SET else set(fns) - strip)
        for name, fns in t.items()
    }


bacc.get_activation_tables = _patched_get_act_tables

B, S, D, H = 4, 2048, 512, 8
P = 128
DC = D // P          # head/model dim chunks (4)
KC = S // P          # k chunks per batch (16)
QB = 512             # q-block size
NQB = S // QB        # q blocks per batch (4)
EPS = 1e-5
F32 = mybir.dt.float32
F32R = mybir.dt.float32r
AF = mybir.ActivationFunctionType
ALU = mybir.AluOpType

N_CORES = 8

_CACHE = {}


class _Kern:
    """Holds the pools/constants and emits the per-batch phases."""

    def __init__(self, nc, tc, pools):
        self.nc = nc
        self.tc = tc
        for k, v in pools.items():
            setattr(self, k, v)

    def setup_consts(self, qb_d, kb_d, w_drams):
        nc = self.nc
        self.ident = self.const.tile([P, P], F32, name="ident")
        make_identity(nc, self.ident)
        ones_raw = self.const.tile([P, 1], F32, name="ones_raw")
        nc.vector.memset(ones_raw, 1.0)
        self.ones_r = self.const.tile([P, 1], F32R, name="ones_r")
        nc.vector.tensor_copy(self.ones_r, ones_raw.bitcast(F32R))
        self.eps_t = self.const.tile([P, 1], F32, name="eps_t")
        nc.vector.memset(self.eps_t, EPS)
        self.qb_t = self.const.tile([P, DC], F32, name="qb_t")
        nc.gpsimd.dma_start(out=self.qb_t,
                            in_=qb_d.rearrange("(c p) -> p c", p=P))
        self.kb_t = self.const.tile([P, DC], F32, name="kb_t")
        nc.gpsimd.dma_start(out=self.kb_t,
                            in_=kb_d.rearrange("(c p) -> p c", p=P))
        # weights: load (SWDGE queue, so x loads aren't stuck behind) and
        # round to f32r
        self.w_r = {}
        for n, dram in w_drams.items():
            wst = self.big.tile([P, DC, D], F32, name=f"{n}_stage", tag="big")
            nc.gpsimd.dma_start(out=wst,
                                in_=dram.rearrange("(c p) n -> p c n", p=P))
            self.w_r[n] = self.wts.tile([P, DC, D], F32R, name=f"{n}_r", tag=n)
            nc.scalar.copy(self.w_r[n], wst.bitcast(F32R))

    # ---- phase A: LayerNorm + transpose -> xhT [d, r] ----
    def phase_a(self, x, b):
        nc = self.nc
        xhT = self.big.tile([P, DC, S], F32R, name=f"xhT{b}", tag="big")
        for g in range(KC // 2):
            xg = self.stage.tile([P, 2, D], F32, name="xg", tag="xg", bufs=2)
            r0 = g * 2 * P
            nc.sync.dma_start(
                out=xg,
                in_=x[b, r0:r0 + 2 * P, :].rearrange("(j p) d -> p j d", p=P))
            mvs, rstds = [], []
            for j in range(2):
                st6 = self.stats.tile([P, 6], F32, name="st6", tag=f"st6{j}")
                nc.vector.bn_stats(out=st6, in_=xg[:, j, :])
                mv = self.stats.tile([P, 2], F32, name="mv", tag=f"mv{j}")
                nc.vector.bn_aggr(out=mv, in_=st6)
                mvs.append(mv)
            for j in range(2):
                # rstd = exp(-0.5 * ln(var + eps))
                lnv = self.stats.tile([P, 1], F32, name="lnv", tag=f"lnv{j}")
                nc.scalar.activation(out=lnv, in_=mvs[j][:, 1:2], func=AF.Ln,
                                     bias=self.eps_t)
                rstd = self.stats.tile([P, 1], F32, name="rstd", tag=f"rstd{j}")
                nc.scalar.activation(out=rstd, in_=lnv, func=AF.Exp,
                                     scale=-0.5)
                rstds.append(rstd)
            for j in range(2):
                rt = g * 2 + j
                xh = self.stage.tile([P, D], F32, name="xh", tag="xh", bufs=2)
                nc.vector.tensor_scalar(out=xh, in0=xg[:, j, :],
                                        scalar1=mvs[j][:, 0:1],
                                        scalar2=rstds[j],
                                        op0=ALU.subtract, op1=ALU.mult)
                tp = self.psum.tile([P, D], F32, name="tp", tag="s", bufs=2)
                for dc in range(DC):
                    nc.tensor.transpose(tp[:, dc * P:(dc + 1) * P],
                                        xh[:, dc * P:(dc + 1) * P], self.ident)
                nc.scalar.copy(
                    out=xhT[:, :, rt * P:(rt + 1) * P],
                    in_=tp.rearrange("p (c r) -> p c r", c=DC).bitcast(F32R))
        return xhT

    # ---- phase B: k^T and v projections (full batch) ----
    def phase_b(self, xhT, b):
        nc = self.nc
        kT = self.kv.tile([P, DC, S], F32R, name=f"kT{b}", tag="kT")
        for cc in range(DC):
            for hf in range(2):
                kps = self.psum.tile([P, 2, QB], F32, name="kps", tag="s",
                                     bufs=2)
                for dc in range(DC):
                    for j in range(2):
                        q0 = (hf * 2 + j) * QB
                        nc.tensor.matmul(
                            kps[:, j, :],
                            self.w_r["kw"][:, dc, cc * P:(cc + 1) * P],
                            xhT[:, dc, q0:q0 + QB],
                            start=(dc == 0), stop=(dc == DC - 1))
                nc.vector.tensor_scalar_add(
                    out=kT[:, cc, hf * 2 * QB:(hf + 1) * 2 * QB],
                    in0=kps.rearrange("p j q -> p (j q)").bitcast(F32R),
                    scalar1=self.kb_t[:, cc:cc + 1])
        v_t = self.kv.tile([P, KC, D], F32R, name=f"v{b}", tag="v")
        for rp in range(KC // 2):
            vps = self.psum.tile([P, 2, D], F32, name="vps", tag="s", bufs=2)
            for dc in range(DC):
                for j in range(2):
                    rc = rp * 2 + j
                    nc.tensor.matmul(
                        vps[:, j, :], xhT[:, dc, rc * P:(rc + 1) * P],
                        self.w_r["vw"][:, dc, :],
                        start=(dc == 0), stop=(dc == DC - 1))
            nc.vector.tensor_copy(out=v_t[:, rp * 2:rp * 2 + 2, :],
                                  in_=vps.bitcast(F32R))
        return kT, v_t

    # ---- q^T projection for one q-block ----
    def qproj(self, xhT, qb_i):
        nc = self.nc
        q0 = qb_i * QB
        qT = self.qtp.tile([P, DC, QB], F32R, name=f"qT{qb_i}", tag="qT")
        for cp in range(DC // 2):
            qps = self.psum.tile([P, 2, QB], F32, name="qps", tag="s", bufs=2)
            for dc in range(DC):
                for j in range(2):
                    cc = cp * 2 + j
                    nc.tensor.matmul(
                        qps[:, j, :],
                        self.w_r["qw"][:, dc, cc * P:(cc + 1) * P],
                        xhT[:, dc, q0:q0 + QB],
                        start=(dc == 0), stop=(dc == DC - 1))
            for j in range(2):
                cc = cp * 2 + j
                nc.vector.tensor_scalar_add(out=qT[:, cc, :],
                                            in0=qps[:, j, :].bitcast(F32R),
                                            scalar1=self.qb_t[:, cc:cc + 1])
        return qT

    # ---- attention scores: S^T + exp for one q-block ----
    def attn_scores(self, qT, kT):
        nc = self.nc
        pT = self.big.tile([P, KC, QB], F32R, name="pT", tag="big")
        for kp in range(KC // 2):
            sps = self.psum.tile([P, 2, QB], F32, name="sps", tag="s", bufs=2)
            for dc in range(DC):
                for j in range(2):
                    kc = kp * 2 + j
                    nc.tensor.matmul(
                        sps[:, j, :], kT[:, dc, kc * P:(kc + 1) * P],
                        qT[:, dc, :],
                        start=(dc == 0), stop=(dc == DC - 1))
            nc.scalar.activation(out=pT[:, kp * 2:kp * 2 + 2, :],
                                 in_=sps.bitcast(F32R), func=AF.Exp)
        return pT

    # ---- attention l + att@V for one q-block ----
    def attn_av(self, lsum, pT, v_t, b, qb_i, skip_av=False):
        nc = self.nc
        q0 = qb_i * QB
        l_ps = self.psum.tile([1, QB], F32, name="l_ps", tag="ly", bufs=2)
        for kc in range(KC):
            nc.tensor.matmul(l_ps, self.ones_r, pT[:, kc, :],
                             start=(kc == 0), stop=(kc == KC - 1))
        l_sb = self.lsbp.tile([1, QB], F32, name="l_sb", tag="l")
        nc.vector.tensor_copy(out=l_sb, in_=l_ps)
        nc.sync.dma_start(out=lsum[b, q0:q0 + QB].unsqueeze(0), in_=l_sb)
        if skip_av:
            return None
        oT = self.otp.tile([P, DC, QB], F32R, name="oT", tag="oT")
        for dc in range(DC):
            o_ps = self.psum.tile([P, QB], F32, name="o_ps", tag="o", bufs=2)
            for kc in range(KC):
                nc.tensor.matmul(o_ps, v_t[:, kc, dc * P:(dc + 1) * P],
                                 pT[:, kc, :],
                                 start=(kc == 0), stop=(kc == KC - 1))
            nc.scalar.copy(out=oT[:, dc, :], in_=o_ps.bitcast(F32R))
        return oT

    # ---- attention tail: output projection + store ----
    def attn_tail(self, y, oT, b, qb_i):
        nc = self.nc
        q0 = qb_i * QB
        for qc in range(QB // P):
            yps = self.psum.tile([P, D], F32, name="yps", tag="ly", bufs=2)
            for dc in range(DC):
                nc.tensor.matmul(yps, oT[:, dc, qc * P:(qc + 1) * P],
                                 self.w_r["ow"][:, dc, :],
                                 start=(dc == 0), stop=(dc == DC - 1))
            yt = self.stage.tile([P, D], F32, name="yt", tag="yt", bufs=3)
            nc.vector.tensor_copy(out=yt, in_=yps)
            r0 = q0 + qc * P
            nc.sync.dma_start(out=y[b, r0:r0 + P, :], in_=yt)


def build(repeat=None, phases="full"):
    """repeat=R wraps the whole compute in a hardware For_i loop that runs
    it R times -- used only for wall-clock device-time benchmarking.
    phases in {"A", "AB", "ABS", "full"} truncates the pipeline (bench)."""
    import contextlib

    nc = bacc.Bacc("TRN2", target_bir_lowering=False, debug=False,
                   num_devices=N_CORES)
    x = nc.dram_tensor("x", [B, S, D], F32, kind="ExternalInput").ap()
    w_drams = {
        n: nc.dram_tensor(n, [D, D], F32, kind="ExternalInput").ap()
        for n in ("qw", "kw", "vw", "ow")
    }
    qb_d = nc.dram_tensor("qb", [D], F32, kind="ExternalInput").ap()
    kb_d = nc.dram_tensor("kb", [D], F32, kind="ExternalInput").ap()
    y = nc.dram_tensor("y", [B, S, D], F32, kind="ExternalOutput").ap()
    lsum = nc.dram_tensor("lsum", [B, S], F32, kind="ExternalOutput").ap()

    with tile.TileContext(nc) as tc:
        with (
            tc.tile_pool(name="const", bufs=1) as const,
            tc.tile_pool(name="wts", bufs=1) as wts,
            tc.tile_pool(name="kv", bufs=1) as kv,
            tc.tile_pool(name="big", bufs=2) as big,
            tc.tile_pool(name="qt", bufs=2) as qtp,
            tc.tile_pool(name="ot", bufs=1) as otp,
            tc.tile_pool(name="stage", bufs=1) as stage,
            tc.tile_pool(name="stats", bufs=4) as stats,
            tc.tile_pool(name="lsb", bufs=1) as lsbp,
            tc.tile_pool(name="psum", bufs=1, space="PSUM") as psum,
        ):
            k = _Kern(nc, tc, dict(const=const, wts=wts, kv=kv, big=big,
                                   qtp=qtp, otp=otp, stage=stage, stats=stats,
                                   lsbp=lsbp, psum=psum))
            k.setup_consts(qb_d, kb_d, w_drams)

            loop_cm = (tc.For_i(0, repeat, 1) if repeat
                       else contextlib.nullcontext())
            with loop_cm:
                xhT = k.phase_a(x, 0)
                for b in range(B):
                    nxt = None
                    if phases == "A":
                        if b + 1 < B:
                            nxt = k.phase_a(x, b + 1)
                        xhT = nxt
                        continue
                    kT, v_t = k.phase_b(xhT, b)
                    if phases == "AB":
                        if b + 1 < B:
                            nxt = k.phase_a(x, b + 1)
                        xhT = nxt
                        continue
                    skip_av = phases == "ABS"
                    qT0 = k.qproj(xhT, 0)
                    qT1 = k.qproj(xhT, 1)
                    pT0 = k.attn_scores(qT0, kT)
                    oT0 = k.attn_av(lsum, pT0, v_t, b, 0, skip_av)
                    if not skip_av:
                        k.attn_tail(y, oT0, b, 0)
                    qT2 = k.qproj(xhT, 2)
                    pT1 = k.attn_scores(qT1, kT)
                    qT3 = k.qproj(xhT, 3)
                    oT1 = k.attn_av(lsum, pT1, v_t, b, 1, skip_av)
                    # hoist next batch's LayerNorm into this batch's att@V
                    # window (xhT slot freed by qproj(3))
                    if b + 1 < B:
                        nxt = k.phase_a(x, b + 1)
                    if not skip_av:
                        k.attn_tail(y, oT1, b, 1)
                    pT2 = k.attn_scores(qT2, kT)
                    oT2 = k.attn_av(lsum, pT2, v_t, b, 2, skip_av)
                    if not skip_av:
                        k.attn_tail(y, oT2, b, 2)
                    pT3 = k.attn_scores(qT3, kT)
                    oT3 = k.attn_av(lsum, pT3, v_t, b, 3, skip_av)
                    if not skip_av:
                        k.attn_tail(y, oT3, b, 3)
                    xhT = nxt

    nc.compile()
    return nc


def _prep_core_inputs(inputs, h):
    """Fold LN affine + attention scale into per-head weights (float64)."""
    x = np.asarray(inputs["x"], np.float32)
    ln_w = np.asarray(inputs["ln_w"], np.float64)
    ln_b = np.asarray(inputs["ln_b"], np.float64)
    sl = slice(h * D, (h + 1) * D)
    scale = float(D) ** -0.5
    q_w = np.asarray(inputs["q_w"], np.float64)[:, sl]
    k_w = np.asarray(inputs["k_w"], np.float64)[:, sl]
    v_w = np.asarray(inputs["v_w"], np.float64)[:, sl]
    o_w = np.asarray(inputs["o_w"], np.float64)[sl, :]
    q_b = np.asarray(inputs["q_b"], np.float64)[sl]
    k_b = np.asarray(inputs["k_b"], np.float64)[sl]
    qw = (ln_w[:, None] * q_w) * scale
    kw = ln_w[:, None] * k_w
    vw = ln_w[:, None] * v_w
    qb = (ln_b @ q_w + q_b) * scale
    kb = ln_b @ k_w + k_b
    return {
        "x": x,
        "qw": qw.astype(np.float32), "kw": kw.astype(np.float32),
        "vw": vw.astype(np.float32), "ow": o_w.astype(np.float32),
        "qb": qb.astype(np.float32), "kb": kb.astype(np.float32),
    }


def kernel(**inputs):
    if "nc" not in _CACHE:
        _CACHE["nc"] = build()
    nc = _CACHE["nc"]

    in_maps = [_prep_core_inputs(inputs, h) for h in range(N_CORES)]
    res = run_bass_kernel_spmd(nc, in_maps, core_ids=list(range(N_CORES)))

    out = np.zeros((B, S, D), np.float64)
    for h in range(N_CORES):
        yh = res.results[h]["y"].astype(np.float64)
        lh = res.results[h]["lsum"].astype(np.float64)
        out += yh / lh[..., None]

    # host-folded constant row: sum_h vb_h @ ow_h + o_b
    ln_b = np.asarray(inputs["ln_b"], np.float64)
    v_w = np.asarray(inputs["v_w"], np.float64)
    v_b = np.asarray(inputs["v_b"], np.float64)
    o_w = np.asarray(inputs["o_w"], np.float64)
    o_b = np.asarray(inputs["o_b"], np.float64)
    vb_full = ln_b @ v_w + v_b            # [D*H]
    out += vb_full @ o_w + o_b
    return out.astype(np.float32)



# revision 12
# speedup vs baseline: 1.3138x; 1.3138x over previous
"""Trainium2 Bass kernel for nn_Attention_48206712930624.

Dense transformer block: LayerNorm -> QKV proj -> 8-head attention
(head_dim = 512) -> output projection.  B=4, S=2048, D=512, H=8.

Sharding: tensor-parallel over heads -- each of the 8 NeuronCores computes
one head end-to-end (LN duplicated), producing a partial output projection
Y_h = (P_h @ V_h) @ o_w_h (un-normalized) plus the softmax denominators
l_h.  The host combines:  out = sum_h Y_h / l_h + const.

Device-side design notes:
  * The whole matmul datapath runs in bf16 (weights quantized on the host,
    activations converted at the existing PSUM-evacuation points) with fp32
    PSUM accumulation.  bf16 keeps the PE at full stream rate (1 col/cyc,
    same as f32r) but halves LDWEIGHTS time via fast-weight-load, halves
    SBUF traffic, doubles DVE elementwise rate, and makes PE transposes
    1 cyc/row instead of 2.  Measured rel err ~1e-3 vs the 2e-2 gate.
  * Scores are computed TRANSPOSED (S^T[k,q] = k^T.T @ q^T) so softmax's
    exp is orientation-agnostic (ACT elementwise) and P^T lands directly
    in the layout att@V needs (k on partitions).  Row sums l are computed
    with a ones-vector matmul.  No max-subtraction (logits ~ N(0,1);
    folded scale keeps exp well within range).
  * LayerNorm scale/bias and the attention scale are folded into the
    weights on the host; v-bias and o_b fold into a constant row added on
    the host.  rstd = exp(-0.5*ln(var+eps)) keeps the whole kernel on ONE
    ACT table set (natural_log_exp_and_others).
  * LayerNorm is split into two passes so the PE never head-of-line
    blocks on the LN chain: a stats pass (DMA + bn_stats + normalize ->
    xh_all, DVE/ACT only) emitted early in the PREVIOUS batch's attention,
    and a transpose pass (PE) emitted at the old hoist point, by which
    time xh_all has long been ready.
  * Scores use paired PSUM banks so each exp covers [128,1024].
"""

import sys

import numpy as np

for _p in ("/opt/trn_rl_repo", "/root/.axon_site/_ro/trn_rl_repo"):
    if _p not in sys.path:
        sys.path.append(_p)

import concourse.bacc as bacc
import concourse.mybir as mybir
import concourse.tile as tile
from concourse.bass_utils import run_bass_kernel_spmd
from concourse.masks import make_identity

# Steer the ACT-table-load placement pass to the one set that holds every
# function this kernel uses (ln, exp, copy, identity), so the whole kernel
# runs on a single table load instead of thrashing between per-function
# sets.  Only the pass's view is doctored; runtime tables are untouched.
_ONE_SET = "natural_log_exp_and_others"
_orig_get_act_tables = bacc.get_activation_tables


def _patched_get_act_tables(arch):
    t = _orig_get_act_tables(arch)
    af = mybir.ActivationFunctionType
    strip = {af.Ln, af.Exp, af.Copy, af.Identity}
    return {
        name: (set(fns) if name == _ONE_SET else set(fns) - strip)
        for name, fns in t.items()
    }


bacc.get_activation_tables = _patched_get_act_tables

B, S, D, H = 4, 2048, 512, 8
P = 128
DC = D // P          # head/model dim chunks (4)
KC = S // P          # k chunks per batch (16)
QB = 512             # q-block size
NQB = S // QB        # q blocks per batch (4)
EPS = 1e-5
F32 = mybir.dt.float32
BF16 = mybir.dt.bfloat16
AF = mybir.ActivationFunctionType
ALU = mybir.AluOpType

N_CORES = 8

_CACHE = {}


class _Kern:
    """Holds the pools/constants and emits the per-batch phases."""

    def __init__(self, nc, tc, pools):
        self.nc = nc
        self.tc = tc
        for k, v in pools.items():
            setattr(self, k, v)

    def setup_consts(self, qb_d, kb_d, w_drams):
        nc = self.nc
        self.ident = self.const.tile([P, P], BF16, name="ident")
        make_identity(nc, self.ident)
        self.ones_b = self.const.tile([P, 1], BF16, name="ones_b")
        nc.vector.memset(self.ones_b, 1.0)
        self.eps_t = self.const.tile([P, 1], F32, name="eps_t")
        nc.vector.memset(self.eps_t, EPS)
        self.qb_t = self.const.tile([P, DC], F32, name="qb_t")
        nc.gpsimd.dma_start(out=self.qb_t,
                            in_=qb_d.rearrange("(c p) -> p c", p=P))
        self.kb_t = self.const.tile([P, DC], F32, name="kb_t")
        nc.gpsimd.dma_start(out=self.kb_t,
                            in_=kb_d.rearrange("(c p) -> p c", p=P))
        # weights arrive pre-quantized to bf16 from the host: DMA directly,
        # no staging or conversion pass
        self.w = {}
        for n, dram in w_drams.items():
            self.w[n] = self.wts.tile([P, DC, D], BF16, name=f"{n}_t", tag=n)
            nc.gpsimd.dma_start(out=self.w[n],
                                in_=dram.rearrange("(c p) n -> p c n", p=P))

    # ---- phase A1: x DMA issue for a batch (both DMA queues) ----
    def lnx_dma(self, x, b):
        nc = self.nc
        xgs = []
        for g in range(KC // 2):
            xg = self.stage.tile([P, 2, D], BF16, name="xg", tag=f"xg{g}",
                                 bufs=1)
            r0 = g * 2 * P
            q = nc.sync if g % 2 == 0 else nc.gpsimd
            q.dma_start(
                out=xg,
                in_=x[b, r0:r0 + 2 * P, :].rearrange("(j p) d -> p j d", p=P))
            xgs.append(xg)
        return xgs

    # ---- phase A2: LN stats + normalize for a pair of row-chunks ----
    def ln_stats(self, xh_all, xgs, g):
        nc = self.nc
        xg = xgs[g]
        mvs, rstds = [], []
        for j in range(2):
            st6 = self.stats.tile([P, 6], F32, name="st6", tag=f"st6{j}")
            nc.vector.bn_stats(out=st6, in_=xg[:, j, :])
            mv = self.stats.tile([P, 2], F32, name="mv", tag=f"mv{j}")
            nc.vector.bn_aggr(out=mv, in_=st6)
            mvs.append(mv)
        for j in range(2):
            # rstd = exp(-0.5 * ln(var + eps))
            lnv = self.stats.tile([P, 1], F32, name="lnv", tag=f"lnv{j}")
            nc.scalar.activation(out=lnv, in_=mvs[j][:, 1:2], func=AF.Ln,
                                 bias=self.eps_t)
            rstd = self.stats.tile([P, 1], F32, name="rstd", tag=f"rstd{j}")
            nc.scalar.activation(out=rstd, in_=lnv, func=AF.Exp, scale=-0.5)
            rstds.append(rstd)
        for j in range(2):
            rt = g * 2 + j
            nc.vector.tensor_scalar(out=xh_all[:, rt, :], in0=xg[:, j, :],
                                    scalar1=mvs[j][:, 0:1],
                                    scalar2=rstds[j],
                                    op0=ALU.subtract, op1=ALU.mult)

    def new_xh_all(self, b):
        tag, bufs = ("xha0", 1) if b == 0 else ("xha", 1)
        return self.xha.tile([P, KC, D], BF16, name=f"xh{b}", tag=tag,
                             bufs=bufs)

    # ---- phase A3: transpose xh_all -> xhT [d, r] (PE) ----
    def new_xhT(self, b):
        tag, bufs = ("xhT0", 1) if b == 0 else ("xhT", 1)
        return self.big.tile([P, DC, S], BF16, name=f"xhT{b}", tag=tag,
                             bufs=bufs)

    def phase_a_tr(self, xh_all, b, xhT=None, rts=None):
        nc = self.nc
        if xhT is None:
            xhT = self.new_xhT(b)
        for rt in (range(KC) if rts is None else rts):
            tp = self.psum.tile([P, D], BF16, name="tp", tag="s", bufs=2)
            for dc in range(DC):
                nc.tensor.transpose(tp[:, dc * P:(dc + 1) * P],
                                    xh_all[:, rt, dc * P:(dc + 1) * P],
                                    self.ident)
            nc.scalar.copy(
                out=xhT[:, :, rt * P:(rt + 1) * P],
                in_=tp.rearrange("p (c r) -> p c r", c=DC))
        return xhT

    # ---- phase B: k^T and v projections (full batch) ----
    def new_kT(self, b):
        return self.kv.tile([P, DC, S], BF16, name=f"kT{b}", tag="kT")

    def phase_b_kT_half(self, xhT, kT, hf):
        nc = self.nc
        for cc in range(DC):
            kps = self.psum.tile([P, 2, QB], F32, name="kps", tag="s",
                                 bufs=2)
            for dc in range(DC):
                for j in range(2):
                    q0 = (hf * 2 + j) * QB
                    nc.tensor.matmul(
                        kps[:, j, :],
                        self.w["kw"][:, dc, cc * P:(cc + 1) * P],
                        xhT[:, dc, q0:q0 + QB],
                        start=(dc == 0), stop=(dc == DC - 1))
            nc.vector.tensor_scalar_add(
                out=kT[:, cc, hf * 2 * QB:(hf + 1) * 2 * QB],
                in0=kps.rearrange("p j q -> p (j q)"),
                scalar1=self.kb_t[:, cc:cc + 1])

    def phase_b_v(self, xhT, b):
        nc = self.nc
        v_t = self.kv.tile([P, KC, D], BF16, name=f"v{b}", tag="v")
        for rp in range(KC // 2):
            vps = self.psum.tile([P, 2, D], F32, name="vps", tag="s", bufs=2)
            for dc in range(DC):
                for j in range(2):
                    rc = rp * 2 + j
                    nc.tensor.matmul(
                        vps[:, j, :], xhT[:, dc, rc * P:(rc + 1) * P],
                        self.w["vw"][:, dc, :],
                        start=(dc == 0), stop=(dc == DC - 1))
            nc.vector.tensor_copy(out=v_t[:, rp * 2:rp * 2 + 2, :],
                                  in_=vps)
        return v_t

    # ---- q^T projection for one q-block ----
    def qproj(self, xhT, qb_i):
        nc = self.nc
        q0 = qb_i * QB
        qT = self.qtp.tile([P, DC, QB], BF16, name=f"qT{qb_i}", tag="qT")
        for cp in range(DC // 2):
            qps = self.psum.tile([P, 2, QB], F32, name="qps", tag="s", bufs=2)
            for dc in range(DC):
                for j in range(2):
                    cc = cp * 2 + j
                    nc.tensor.matmul(
                        qps[:, j, :],
                        self.w["qw"][:, dc, cc * P:(cc + 1) * P],
                        xhT[:, dc, q0:q0 + QB],
                        start=(dc == 0), stop=(dc == DC - 1))
            for j in range(2):
                cc = cp * 2 + j
                nc.vector.tensor_scalar_add(out=qT[:, cc, :],
                                            in0=qps[:, j, :],
                                            scalar1=self.qb_t[:, cc:cc + 1])
        return qT

    # ---- attention scores: S^T + exp for one q-block ----
    def attn_scores(self, qT, kT):
        nc = self.nc
        pT = self.big.tile([P, KC, QB], BF16, name="pT", tag="pT")
        for kp in range(KC // 2):
            sps = self.psum.tile([P, 2, QB], F32, name="sps", tag="s", bufs=2)
            for dc in range(DC):
                for j in range(2):
                    kc = kp * 2 + j
                    nc.tensor.matmul(
                        sps[:, j, :], kT[:, dc, kc * P:(kc + 1) * P],
                        qT[:, dc, :],
                        start=(dc == 0), stop=(dc == DC - 1))
            nc.scalar.activation(out=pT[:, kp * 2:kp * 2 + 2, :],
                                 in_=sps, func=AF.Exp)
        return pT

    # ---- attention l + att@V for one q-block ----
    def attn_av(self, lsum, pT, v_t, b, qb_i, skip_av=False):
        nc = self.nc
        q0 = qb_i * QB
        l_ps = self.psum.tile([1, QB], F32, name="l_ps", tag="ly", bufs=2)
        for kc in range(KC):
            nc.tensor.matmul(l_ps, self.ones_b, pT[:, kc, :],
                             start=(kc == 0), stop=(kc == KC - 1))
        l_sb = self.lsbp.tile([1, QB], F32, name="l_sb", tag="l")
        nc.vector.tensor_copy(out=l_sb, in_=l_ps)
        nc.sync.dma_start(out=lsum[b, q0:q0 + QB].unsqueeze(0), in_=l_sb)
        if skip_av:
            return None
        oT = self.otp.tile([P, DC, QB], BF16, name="oT", tag="oT")
        for dc in range(DC):
            o_ps = self.psum.tile([P, QB], F32, name="o_ps", tag="o", bufs=2)
            for kc in range(KC):
                nc.tensor.matmul(o_ps, v_t[:, kc, dc * P:(dc + 1) * P],
                                 pT[:, kc, :],
                                 start=(kc == 0), stop=(kc == KC - 1))
            nc.scalar.copy(out=oT[:, dc, :], in_=o_ps)
        return oT

    # ---- attention tail: output projection + store ----
    def attn_tail(self, y, oT, b, qb_i):
        nc = self.nc
        q0 = qb_i * QB
        for qc in range(QB // P):
            yps = self.psum.tile([P, D], F32, name="yps", tag="ly", bufs=2)
            for dc in range(DC):
                nc.tensor.matmul(yps, oT[:, dc, qc * P:(qc + 1) * P],
                                 self.w["ow"][:, dc, :],
                                 start=(dc == 0), stop=(dc == DC - 1))
            yt = self.stage.tile([P, D], F32, name="yt", tag="yt", bufs=3)
            nc.vector.tensor_copy(out=yt, in_=yps)
            r0 = q0 + qc * P
            nc.sync.dma_start(out=y[b, r0:r0 + P, :], in_=yt)


def build(repeat=None, phases="full"):
    """repeat=R wraps the whole compute in a hardware For_i loop that runs
    it R times -- used only for wall-clock device-time benchmarking.
    phases in {"A", "AB", "ABS", "full"} truncates the pipeline (bench)."""
    import contextlib

    nc = bacc.Bacc("TRN2", target_bir_lowering=False, debug=False,
                   num_devices=N_CORES)
    x = nc.dram_tensor("x", [B, S, D], BF16, kind="ExternalInput").ap()
    w_drams = {
        n: nc.dram_tensor(n, [D, D], BF16, kind="ExternalInput").ap()
        for n in ("qw", "kw", "vw", "ow")
    }
    qb_d = nc.dram_tensor("qb", [D], F32, kind="ExternalInput").ap()
    kb_d = nc.dram_tensor("kb", [D], F32, kind="ExternalInput").ap()
    y = nc.dram_tensor("y", [B, S, D], F32, kind="ExternalOutput").ap()
    lsum = nc.dram_tensor("lsum", [B, S], F32, kind="ExternalOutput").ap()

    with tile.TileContext(nc) as tc:
        with (
            tc.tile_pool(name="const", bufs=1) as const,
            tc.tile_pool(name="wts", bufs=1) as wts,
            tc.tile_pool(name="kv", bufs=1) as kv,
            tc.tile_pool(name="xha", bufs=1) as xha,
            tc.tile_pool(name="big", bufs=2) as big,
            tc.tile_pool(name="qt", bufs=2) as qtp,
            tc.tile_pool(name="ot", bufs=1) as otp,
            tc.tile_pool(name="stage", bufs=1) as stage,
            tc.tile_pool(name="stats", bufs=4) as stats,
            tc.tile_pool(name="lsb", bufs=1) as lsbp,
            tc.tile_pool(name="psum", bufs=1, space="PSUM") as psum,
        ):
            k = _Kern(nc, tc, dict(const=const, wts=wts, kv=kv, xha=xha,
                                   big=big, qtp=qtp, otp=otp, stage=stage,
                                   stats=stats, lsbp=lsbp, psum=psum))
            k.setup_consts(qb_d, kb_d, w_drams)

            loop_cm = (tc.For_i(0, repeat, 1) if repeat
                       else contextlib.nullcontext())
            with loop_cm:
                # Body-top prologue: batch 0's LN interleaved per-group with
                # the kT projection halves, so the PE gets work as soon as
                # the first x groups land.  (For_i barriers between
                # iterations, so this serial chain cannot hide behind the
                # previous iteration -- minimize it instead.)
                xh0 = k.new_xh_all(0)
                xgs0 = k.lnx_dma(x, 0)
                xhT = k.new_xhT(0)
                kT0 = k.new_kT(0)
                for g in range(KC // 4):
                    k.ln_stats(xh0, xgs0, g)
                    k.phase_a_tr(xh0, 0, xhT=xhT, rts=(2 * g, 2 * g + 1))
                k.phase_b_kT_half(xhT, kT0, 0)
                for g in range(KC // 4, KC // 2):
                    k.ln_stats(xh0, xgs0, g)
                    k.phase_a_tr(xh0, 0, xhT=xhT, rts=(2 * g, 2 * g + 1))
                k.phase_b_kT_half(xhT, kT0, 1)
                for b in range(B):
                    if b == 0:
                        kT = kT0
                        v_t = k.phase_b_v(xhT, 0)
                    else:
                        kT = k.new_kT(b)
                        k.phase_b_kT_half(xhT, kT, 0)
                        k.phase_b_kT_half(xhT, kT, 1)
                        v_t = k.phase_b_v(xhT, b)
                    nxt_xh = None
                    nxt_xgs = None
                    do_hoist = b + 1 < B
                    nb = b + 1
                    # issue next batch's x loads now (done well before the
                    # spread-out stats groups consume them)
                    if do_hoist:
                        nxt_xh = k.new_xh_all(nb)
                        nxt_xgs = k.lnx_dma(x, nb)

                    def stats_grp(i):
                        if nxt_xh is not None and i < KC // 2:
                            k.ln_stats(nxt_xh, nxt_xgs, i)

                    skip_av = phases == "ABS"
                    qT0 = k.qproj(xhT, 0)
                    stats_grp(0)
                    qT1 = k.qproj(xhT, 1)
                    stats_grp(1)
                    pT0 = k.attn_scores(qT0, kT)
                    stats_grp(2)
                    stats_grp(3)
                    oT0 = k.attn_av(lsum, pT0, v_t, b, 0, skip_av)
                    stats_grp(4)
                    stats_grp(5)
                    if not skip_av:
                        k.attn_tail(y, oT0, b, 0)
                    qT2 = k.qproj(xhT, 2)
                    stats_grp(6)
                    pT1 = k.attn_scores(qT1, kT)
                    stats_grp(7)
                    qT3 = k.qproj(xhT, 3)
                    oT1 = k.attn_av(lsum, pT1, v_t, b, 1, skip_av)
                    # hoist next batch's LN transposes into this batch's
                    # att@V window (xhT slot freed by qproj(3); xh_all has
                    # been ready since the stats groups above)
                    if do_hoist:
                        xhT = k.phase_a_tr(nxt_xh, nb)
                    else:
                        xhT = None
                    if not skip_av:
                        k.attn_tail(y, oT1, b, 1)
                    pT2 = k.attn_scores(qT2, kT)
                    oT2 = k.attn_av(lsum, pT2, v_t, b, 2, skip_av)
                    if not skip_av:
                        k.attn_tail(y, oT2, b, 2)
                    pT3 = k.attn_scores(qT3, kT)
                    oT3 = k.attn_av(lsum, pT3, v_t, b, 3, skip_av)
                    if not skip_av:
                        k.attn_tail(y, oT3, b, 3)

    nc.compile()
    return nc


def _prep_core_inputs(inputs, h):
    """Fold LN affine + attention scale into per-head weights (float64),
    then quantize the weights to bf16 for the device."""
    bf = mybir.dt.np(BF16)
    x = np.asarray(inputs["x"], np.float32).astype(bf)
    ln_w = np.asarray(inputs["ln_w"], np.float64)
    ln_b = np.asarray(inputs["ln_b"], np.float64)
    sl = slice(h * D, (h + 1) * D)
    scale = float(D) ** -0.5
    q_w = np.asarray(inputs["q_w"], np.float64)[:, sl]
    k_w = np.asarray(inputs["k_w"], np.float64)[:, sl]
    v_w = np.asarray(inputs["v_w"], np.float64)[:, sl]
    o_w = np.asarray(inputs["o_w"], np.float64)[sl, :]
    q_b = np.asarray(inputs["q_b"], np.float64)[sl]
    k_b = np.asarray(inputs["k_b"], np.float64)[sl]
    qw = (ln_w[:, None] * q_w) * scale
    kw = ln_w[:, None] * k_w
    vw = ln_w[:, None] * v_w
    qb = (ln_b @ q_w + q_b) * scale
    kb = ln_b @ k_w + k_b
    return {
        "x": x,
        "qw": qw.astype(bf), "kw": kw.astype(bf),
        "vw": vw.astype(bf), "ow": o_w.astype(bf),
        "qb": qb.astype(np.float32), "kb": kb.astype(np.float32),
    }


def kernel(**inputs):
    if "nc" not in _CACHE:
        _CACHE["nc"] = build()
    nc = _CACHE["nc"]

    in_maps = [_prep_core_inputs(inputs, h) for h in range(N_CORES)]
    res = run_bass_kernel_spmd(nc, in_maps, core_ids=list(range(N_CORES)))

    out = np.zeros((B, S, D), np.float64)
    for h in range(N_CORES):
        yh = res.results[h]["y"].astype(np.float64)
        lh = res.results[h]["lsum"].astype(np.float64)
        out += yh / lh[..., None]

    # host-folded constant row: sum_h vb_h @ ow_h + o_b
    ln_b = np.asarray(inputs["ln_b"], np.float64)
    v_w = np.asarray(inputs["v_w"], np.float64)
    v_b = np.asarray(inputs["v_b"], np.float64)
    o_w = np.asarray(inputs["o_w"], np.float64)
    o_b = np.asarray(inputs["o_b"], np.float64)
    vb_full = ln_b @ v_w + v_b            # [D*H]
    out += vb_full @ o_w + o_b
    return out.astype(np.float32)
